# revision 14
# baseline (speedup 1.0000x reference)
"""LISTA sparse encoder kernel for 8 Trainium2 NeuronCores.

Math (reference):
    x_flat = x.transpose(0,2,3,1).reshape(-1, C)          # [N, C], N = B*H*W
    Wx = W @ x_flat.T                                     # [K, N]
    z = 0
    repeat num_steps:  z = soft_threshold((I - S) @ (z + Wx), thresh)
    out0 = z -> [B, K, H, W];  out1 = dictionary / ||rows||

Rewrite used here: with A = I - S and the iterate y_t = z_t + Wx,
    y_0 = Wx
    u_t = A @ y_{t-1}
    z_t = relu(u_t - th) - relu(-u_t - th)     (= soft_threshold(u_t))
    y_t = z_t + Wx                              (skipped on the last step)
so each step is one K x K GEMM plus cheap elementwise work, and the "+ Wx"
add folds into the iterate instead of a separate pre-GEMM add.

Sharding: data-parallel over pixels. Core i takes batches [4i, 4i+4) ->
4096 columns of the [K, N] problem; A^T, W^T, dictionary are replicated.
"""

import os
import sys

import numpy as np

if "/opt/trn_rl_repo" not in sys.path:
    sys.path.insert(0, "/opt/trn_rl_repo")

from contextlib import ExitStack

import concourse.bass as bass
import concourse.tile as tile
from concourse import bass2jax as _bass2jax
from concourse import bass_utils as _bass_utils
from concourse import mybir
from concourse.bass_utils import run_bass_kernel_spmd

# This walrus build caps DMACopy / TensorScalar pseudo-instructions at ONE
# sync-wait command, but Tile's sem assignment can attach two (producer wait
# + HWDGE queue-credit wait). Split the excess onto standalone EventSemaphore
# instructions on the same engine immediately before the capped instruction —
# engine program order preserves the happens-before.
_WAIT_CAPS = {
    "DMACopy": 1,
    "TensorScalarPtr": 1,
    "TensorScalar": 1,
    "Activation": 1,
    "TensorTensor": 1,
    "Matmult": 1,
    "Ldweights": 1,
    "Memset": 1,
    "Reciprocal": 1,
    "TensorReduce": 1,
    "TensorCopy": 1,
    "Drain": 1,
}


def _split_excess_waits(bir_json: bytes) -> bytes:
    import json as _json

    bir = _json.loads(bir_json)
    changed = False
    uid = [0]
    for fn in bir.get("functions", []):
        for bb in fn.get("blocks", []):
            out = []
            for inst in bb.get("instructions", []):
                cap = _WAIT_CAPS.get(inst.get("opcode"))
                si = inst.get("sync_info") or {}
                waits = si.get("on_wait") or []
                if cap is not None and len(waits) > cap:
                    changed = True
                    for w in waits:
                        uid[0] += 1
                        ev = {
                            "engine": inst.get("engine"),
                            "ins": [],
                            "name": f"waitsplit_{uid[0]}_{inst.get('name')}",
                            "opcode": "EventSemaphore",
                            "outs": [],
                            "sync_info": {"on_update": [], "on_wait": [w]},
                        }
                        if "debug" in inst:
                            ev["debug"] = inst["debug"]
                        out.append(ev)
                    si["on_wait"] = []
                    inst["sync_info"] = si
                out.append(inst)
            bb["instructions"] = out
    if not changed:
        return bir_json
    return _json.dumps(bir).encode()


_orig_compile_bir_kernel = _bass_utils.compile_bir_kernel


def _patched_compile_bir_kernel(bir_json, tmpdir, neff_name="file.neff"):
    return _orig_compile_bir_kernel(_split_excess_waits(bir_json), tmpdir, neff_name)


_bass_utils.compile_bir_kernel = _patched_compile_bir_kernel
_bass2jax.compile_bir_kernel = _patched_compile_bir_kernel

# Problem constants (hardcoded per the task contract).
B, C, H, Wd = 32, 512, 32, 32
K = 1024          # sparse_dim
NCORES = 8
BL = B // NCORES  # batches per core
N_LOC = BL * H * Wd   # 4096 columns per core
CHUNK = 512           # free-dim tile (one PSUM bank)
NCHUNK = N_LOC // CHUNK
P = 128
KT = K // P       # 8 k/m tiles
CT = C // P       # 4 c tiles

F16 = mybir.dt.float16
F32 = mybir.dt.float32
AF = mybir.ActivationFunctionType

LAST_RESULTS = None  # stashed BassKernelResults for test harnesses

_PROGRAM_CACHE = {}


def _build_program(num_steps: int) -> bass.Bass:
    nc = bass.Bass()

    at_d = nc.declare_dram_parameter("at", [K, K], F16, isOutput=False)
    wt_d = nc.declare_dram_parameter("wt", [C, K], F16, isOutput=False)
    xt_d = nc.declare_dram_parameter("xt", [C, N_LOC], F16, isOutput=False)
    negthr_d = nc.declare_dram_parameter("negthr", [P, 1], F32, isOutput=False)
    dict_d = nc.declare_dram_parameter("dict_in", [K, C], F32, isOutput=False)
    z_d = nc.declare_dram_parameter("z_out", [BL, K, H * Wd], F32, isOutput=True)
    dn_d = nc.declare_dram_parameter("dnorm", [K, C], F32, isOutput=True)

    with TileCtx(nc) as tc, ExitStack() as ctx:
        const_pool = ctx.enter_context(tc.tile_pool(name="const", bufs=1))
        negthr = const_pool.tile([P, 1], F32)
        nc.sync.dma_start(negthr[:], negthr_d[:])

        # --- weights / activations in ---
        w_pool = ctx.enter_context(tc.tile_pool(name="wts", bufs=1))
        at_sb = []
        for k in range(KT):
            at_t = w_pool.tile([P, K], F16, tag=f"at{k}")
            nc.sync.dma_start(at_t[:], at_d[k * P : (k + 1) * P, :])
            at_sb.append(at_t)
        wt_sb = []
        xt_sb = []
        for c in range(CT):
            wt_t = w_pool.tile([P, K], F16, tag=f"wt{c}")
            nc.sync.dma_start(wt_t[:], wt_d[c * P : (c + 1) * P, :])
            wt_sb.append(wt_t)
            xt_t = w_pool.tile([P, N_LOC], F16, tag=f"xt{c}")
            nc.sync.dma_start(xt_t[:], xt_d[c * P : (c + 1) * P, :])
            xt_sb.append(xt_t)

        # --- dictionary row-normalize (small; overlaps with main loop) ---
        dpool = ctx.enter_context(tc.tile_pool(name="dict", bufs=2))
        dsm = ctx.enter_context(tc.tile_pool(name="dsm", bufs=4))
        for i in range(KT):
            dt_ = dpool.tile([P, C], F32, tag="din")
            nc.sync.dma_start(dt_[:], dict_d[i * P : (i + 1) * P, :])
            sq = dpool.tile([P, C], F32, tag="dsq")
            ssq = dsm.tile([P, 1], F32, tag="ssq")
            nc.scalar.activation(sq[:], dt_[:], AF.Square, accum_out=ssq[:])
            rec = dsm.tile([P, 1], F32, tag="rec")
            nc.vector.reciprocal(rec[:], ssq[:])
            q = dsm.tile([P, 1], F32, tag="q")
            nc.scalar.sqrt(q[:], rec[:])  # ~rsqrt(ssq)
            # one Newton step: q1 = q * (1.5 - 0.5 * ssq * q^2)
            t0 = dsm.tile([P, 1], F32, tag="t0")
            nc.vector.tensor_mul(t0[:], q[:], q[:])
            t1 = dsm.tile([P, 1], F32, tag="t1")
            nc.vector.tensor_mul(t1[:], t0[:], ssq[:])
            t2 = dsm.tile([P, 1], F32, tag="t2")
            nc.scalar.activation(t2[:], t1[:], AF.Copy, bias=1.5, scale=-0.5)
            q1 = dsm.tile([P, 1], F32, tag="q1")
            nc.vector.tensor_mul(q1[:], t2[:], q[:])
            dn = dpool.tile([P, C], F32, tag="dout")
            nc.scalar.mul(dn[:], dt_[:], q1[:])
            nc.scalar.dma_start(dn_d[i * P : (i + 1) * P, :], dn[:])

        # --- main LISTA loop ---
        psum_pool = ctx.enter_context(tc.tile_pool(name="psum", bufs=8, space="PSUM"))
        wx_pool = ctx.enter_context(tc.tile_pool(name="wx", bufs=2 * KT))
        y_pool = ctx.enter_context(tc.tile_pool(name="y", bufs=4 * KT))
        ab_pool = ctx.enter_context(tc.tile_pool(name="ab", bufs=4))
        zo_pool = ctx.enter_context(tc.tile_pool(name="zo", bufs=4))

        for pair in range(NCHUNK // 2):
            chunks = (2 * pair, 2 * pair + 1)
            wx = {}
            y_prev = {}
            for ch in chunks:
                co = ch * CHUNK
                wx[ch] = []
                for m in range(KT):
                    ps = psum_pool.tile([P, CHUNK], F32, tag="psum")
                    for c in range(CT):
                        nc.tensor.matmul(
                            ps[:],
                            wt_sb[c][:, m * P : (m + 1) * P],
                            xt_sb[c][:, co : co + CHUNK],
                            start=(c == 0),
                            stop=(c == CT - 1),
                        )
                    wxt = wx_pool.tile([P, CHUNK], F16, tag="wx")
                    nc.scalar.copy(wxt[:], ps[:])
                    wx[ch].append(wxt)
                y_prev[ch] = wx[ch]

            for t in range(1, num_steps + 1):
                last = t == num_steps
                for ch in chunks:
                    co = ch * CHUNK
                    b_loc, half = ch // 2, ch % 2
                    y_new = []
                    for m in range(KT):
                        ps = psum_pool.tile([P, CHUNK], F32, tag="psum")
                        for k in range(KT):
                            nc.tensor.matmul(
                                ps[:],
                                at_sb[k][:, m * P : (m + 1) * P],
                                y_prev[ch][k][:],
                                start=(k == 0),
                                stop=(k == KT - 1),
                            )
                        a = ab_pool.tile([P, CHUNK], F32, tag="a")
                        nc.scalar.activation(a[:], ps[:], AF.Relu, bias=negthr[:], scale=1.0)
                        b = ab_pool.tile([P, CHUNK], F32, tag="b")
                        nc.scalar.activation(b[:], ps[:], AF.Relu, bias=negthr[:], scale=-1.0)
                        if last:
                            zt = zo_pool.tile([P, CHUNK], F32, tag="zt")
                            nc.vector.tensor_sub(zt[:], a[:], b[:])
                            nc.gpsimd.dma_start(
                                z_d[
                                    b_loc,
                                    m * P : (m + 1) * P,
                                    half * CHUNK : half * CHUNK + CHUNK,
                                ],
                                zt[:],
                            )
                        else:
                            d = ab_pool.tile([P, CHUNK], F16, tag="d")
                            nc.vector.tensor_sub(d[:], a[:], b[:])
                            yt = y_pool.tile([P, CHUNK], F16, tag="y")
                            nc.vector.tensor_add(yt[:], d[:], wx[ch][m][:])
                            y_new.append(yt)
                    if not last:
                        y_prev[ch] = y_new

    return nc


def TileCtx(nc):
    return tile.TileContext(nc)


def kernel(x, dictionary, W, S, thresh, num_steps):
    global LAST_RESULTS
    num_steps = int(num_steps)
    thresh_f = float(np.asarray(thresh))

    x = np.asarray(x, dtype=np.float32)
    dictionary = np.asarray(dictionary, dtype=np.float32)
    W = np.asarray(W, dtype=np.float32)
    S = np.asarray(S, dtype=np.float32)

    if num_steps <= 0:
        dn = dictionary / np.linalg.norm(dictionary, axis=1, keepdims=True)
        z = np.zeros((B, K, H, Wd), dtype=np.float32)
        return (z, dn)

    if num_steps not in _PROGRAM_CACHE:
        _PROGRAM_CACHE[num_steps] = _build_program(num_steps)
    nc = _PROGRAM_CACHE[num_steps]

    at_np = np.ascontiguousarray((np.eye(K, dtype=np.float32) - S).T).astype(np.float16)
    wt_np = np.ascontiguousarray(W.T).astype(np.float16)
    negthr_np = np.full((P, 1), -thresh_f, dtype=np.float32)

    in_maps = []
    for i in range(NCORES):
        xs = x[i * BL : (i + 1) * BL]  # [BL, C, H, W]
        xt_np = np.ascontiguousarray(
            xs.transpose(1, 0, 2, 3).reshape(C, N_LOC)
        ).astype(np.float16)
        in_maps.append(
            {
                "at": at_np,
                "wt": wt_np,
                "xt": xt_np,
                "negthr": negthr_np,
                "dict_in": dictionary,
            }
        )

    res = run_bass_kernel_spmd(
        nc,
        in_maps,
        core_ids=list(range(NCORES)),
        trace=bool(int(os.environ.get("KERNEL_TRACE", "0"))),
    )
    LAST_RESULTS = res

    z_full = np.concatenate([res.results[i]["z_out"] for i in range(NCORES)], axis=0)
    z_full = z_full.reshape(B, K, H, Wd)
    dnorm = res.results[0]["dnorm"]
    return (z_full, dnorm)
